# revision 1
# baseline (speedup 1.0000x reference)
"""Trainium2 Bass kernel for nn_LogicLayer (differentiable logic-gate layer).

Reference computation:
    a = x[:, idx_a]; b = x[:, idx_b]                  # [B, OUT] gathers
    w = softmax(weights, -1)                          # [OUT, 16]
    out = sum_k w[:, k] * gate_k(a, b)

Every gate value is of the form c0 + c1*a + c2*b + c3*a*b, so
    out[i, j] = W0[j] + W1[j]*a + W2[j]*b + W3[j]*a*b
with W = softmax(weights) @ C, C the [16, 4] gate-coefficient table.

Kernel strategy (data-parallel over batch across 8 cores, 256 rows/core):
  1. softmax+C projection on device -> W0..W3 tiles kept in SBUF
     (heavy reduces on GPSIMD so they overlap the x loads on DMA)
  2. PE-transpose the core's x shard [256, 8192] -> xT [8192, 256] in DRAM
     (stores batched 4 row-blocks per DMA to keep HWDGE off the critical path)
  3. dma_gather rows of xT for idx_a / idx_b (out_dim lands on partitions)
  4. u = W3*a + W2 (ACT), v = W1*a + W0 (DVE ts), t = u*b (DVE/Pool tt)
  5. out = t + v realized directly in PSUM by PE transpose-accumulate
     (two is_transpose matmuls into the same bank), copied back and stored
     in natural [256, 8192] layout.
"""

import numpy as np

# ---------------------------------------------------------------- constants
B_TOT, IN_DIM, OUT_DIM = 2048, 8192, 8192
NCORES = 8

# value = c0 + c1*a + c2*b + c3*ab  for each of the 16 gates
GATE_C = np.array(
    [
        # c0  c1  c2  c3
        [0, 0, 0, 0],    # 0  False
        [0, 0, 0, 1],    # 1  a AND b
        [0, 1, 0, -1],   # 2  a AND NOT b
        [0, 1, 0, 0],    # 3  a
        [0, 0, 1, -1],   # 4  NOT a AND b
        [0, 0, 1, 0],    # 5  b
        [0, 1, 1, -2],   # 6  a XOR b
        [0, 1, 1, -1],   # 7  a OR b
        [1, -1, -1, 1],  # 8  NOT (a OR b)
        [1, -1, -1, 2],  # 9  NOT (a XOR b)
        [1, 0, -1, 0],   # 10 NOT b
        [1, 0, -1, 1],   # 11 a OR NOT b
        [1, -1, 0, 0],   # 12 NOT a
        [1, -1, 0, 1],   # 13 NOT a OR b
        [1, 0, 0, -1],   # 14 NOT (a AND b)
        [1, 0, 0, 0],    # 15 True
    ],
    dtype=np.float32,
)  # [16, 4]


# ---------------------------------------------------------------- device IR
def build_nc(B=B_TOT // NCORES, IN=IN_DIM, OUT=OUT_DIM, NJ=1024):
    """Build the per-core Bass module (SPMD; all cores run the same IR)."""
    import sys

    if "/opt/trn_rl_repo" not in sys.path:
        sys.path.insert(0, "/opt/trn_rl_repo")

    import concourse.tile as tile
    from concourse import bacc, mybir
    from concourse.masks import make_identity
    from contextlib import ExitStack

    f32 = mybir.dt.float32
    i16 = mybir.dt.int16
    PB = B // 128          # batch partition-blocks
    NCH = OUT // NJ        # out_dim chunks
    SLOTS = NJ // 128      # 128-wide j-slots per chunk
    RPT = OUT // 128       # = NCH * SLOTS  (W free dim per partition)
    LCH = min(IN, 2048)    # x load chunk (columns)
    SG = 4                 # xT row-blocks batched per store

    nc = bacc.Bacc("TRN2", target_bir_lowering=False)
    x = nc.declare_dram_parameter("x", [B, IN], f32, isOutput=False)
    wgt = nc.declare_dram_parameter("wgt_shuf", [128, RPT * 16], f32, isOutput=False)
    cg = nc.declare_dram_parameter("cgate", [128, 64], f32, isOutput=False)
    idxa = nc.declare_dram_parameter("idxa16", [128, OUT // 16], i16, isOutput=False)
    idxb = nc.declare_dram_parameter("idxb16", [128, OUT // 16], i16, isOutput=False)
    out = nc.declare_dram_parameter("out", [B, OUT], f32, isOutput=True)

    Ident = mybir.ActivationFunctionType.Identity
    Exp = mybir.ActivationFunctionType.Exp
    MULT = mybir.AluOpType.mult
    ADD = mybir.AluOpType.add

    with tile.TileContext(nc) as tc, ExitStack() as ctx:
        dram = ctx.enter_context(tc.tile_pool(name="dram", bufs=1, space="DRAM"))
        xT = dram.tile([IN, B], f32, name="xT")

        cpool = ctx.enter_context(tc.tile_pool(name="consts", bufs=1))
        xs_stack = ExitStack()
        xs_pool = xs_stack.enter_context(tc.tile_pool(name="xs", bufs=1, side="right"))
        idx_pool = ctx.enter_context(tc.tile_pool(name="idxp", bufs=1))

        # wgt first (small) so the W-phase chain starts immediately,
        # then x shard loads saturate DMA while W-phase compute runs
        cgt = cpool.tile([128, 64], f32, name="cgt")
        nc.sync.dma_start(cgt[:], cg[:])
        wpool = ctx.enter_context(tc.tile_pool(name="wtmp", bufs=2))
        wtile = wpool.tile([128, RPT * 16], f32, name="wtile")
        nc.sync.dma_start(wtile[:], wgt[:])
        xh = {}
        for c0 in range(IN // LCH):
            for h in range(PB):
                xht = xs_pool.tile([128, LCH], f32, name=f"xh{h}_{c0}",
                                   tag=f"xh{h}_{c0}")
                nc.sync.dma_start(xht[:], x[h * 128:(h + 1) * 128,
                                            c0 * LCH:(c0 + 1) * LCH])
                xh[h, c0] = xht
        idxa_sb = idx_pool.tile([128, OUT // 16], i16, name="idxa_sb")
        nc.sync.dma_start(idxa_sb[:], idxa[:])
        idxb_sb = idx_pool.tile([128, OUT // 16], i16, name="idxb_sb")
        nc.sync.dma_start(idxb_sb[:], idxb[:])

        ident = cpool.tile([128, 128], f32, name="ident")
        make_identity(nc, ident[:])

        # ---- W = softmax(weights) @ C, in (q, r) layout: j = r*128 + q ----
        # heavy elementwise on GPSIMD so DVE stays free for phase-A copybacks
        wk = [cpool.tile([128, RPT], f32, name=f"wk{k}") for k in range(4)]
        if True:
            wexp = wpool.tile([128, RPT * 16], f32, name="wexp")
            nc.scalar.activation(wexp[:], wtile[:], Exp)
            wsum = wpool.tile([128, RPT], f32, name="wsum")
            nc.vector.tensor_reduce(
                out=wsum[:],
                in_=wexp[:].rearrange("p (r k) -> p r k", k=16),
                op=ADD,
                axis=mybir.AxisListType.X,
            )
            wrcp = wpool.tile([128, RPT], f32, name="wrcp")
            nc.vector.reciprocal(wrcp[:], wsum[:])
            for k in range(4):
                wtmp = wpool.tile([128, RPT * 16], f32, name="wtmp", tag="wtmp")
                ck_bcast = (
                    cgt[:, k * 16:(k + 1) * 16]
                    .rearrange("p (r k) -> p r k", r=1)
                    .to_broadcast([128, RPT, 16])
                )
                nc.gpsimd.tensor_tensor(
                    out=wtmp[:].rearrange("p (r k) -> p r k", k=16),
                    in0=wexp[:].rearrange("p (r k) -> p r k", k=16),
                    in1=ck_bcast,
                    op=MULT,
                )
                wred = wpool.tile([128, RPT], f32, name="wred", tag="wred")
                nc.vector.tensor_reduce(
                    out=wred[:],
                    in_=wtmp[:].rearrange("p (r k) -> p r k", k=16),
                    op=ADD,
                    axis=mybir.AxisListType.X,
                )
                nc.vector.tensor_tensor(out=wk[k][:], in0=wred[:], in1=wrcp[:],
                                        op=MULT)

        # ---- phase A: transpose x shard into xT (DRAM) ----
        psumT = ctx.enter_context(tc.tile_pool(name="psumT", bufs=4, space="PSUM"))
        stg_pool = ctx.enter_context(tc.tile_pool(name="xstg", bufs=3))
        if True:
            for g in range(IN // (SG * 128)):
                st = stg_pool.tile([128, SG, B], f32, tag="st")
                for i in range(SG):
                    cb = g * SG + i
                    c0, cc = (cb * 128) // LCH, (cb * 128) % LCH
                    pt = psumT.tile([128, B], f32, tag="pt")
                    for h in range(PB):
                        nc.tensor.transpose(
                            pt[:, h * 128:(h + 1) * 128],
                            xh[h, c0][:, cc:cc + 128],
                            ident[:],
                        )
                    if cb % 2 == 0:
                        nc.vector.tensor_copy(st[:, i, :], pt[:])
                    else:
                        nc.scalar.copy(st[:, i, :], pt[:])
                nc.sync.dma_start(
                    xT[g * SG * 128:(g + 1) * SG * 128, :]
                    .rearrange("(i p) b -> p i b", p=128),
                    st[:],
                )

        xs_stack.close()  # release x tiles; phase-B pools reuse the zone

        # ---- phase B: gather + gates + transpose-back ----
        gpool = ctx.enter_context(tc.tile_pool(name="gath", bufs=4))
        uvpool = ctx.enter_context(tc.tile_pool(name="uv", bufs=12))
        psumO = ctx.enter_context(tc.tile_pool(name="psumO", bufs=4, space="PSUM"))
        ostg = ctx.enter_context(tc.tile_pool(name="ostg", bufs=3))
        if True:
            NJ16 = NJ // 16
            for ck in range(NCH):
                ga = gpool.tile([128, SLOTS, B], f32, tag="ga")
                nc.gpsimd.dma_gather(
                    ga[:], xT[:], idxa_sb[:, ck * NJ16:(ck + 1) * NJ16], NJ, NJ, B
                )
                gb = gpool.tile([128, SLOTS, B], f32, tag="gb")
                nc.gpsimd.dma_gather(
                    gb[:], xT[:], idxb_sb[:, ck * NJ16:(ck + 1) * NJ16], NJ, NJ, B
                )
                for cq in range(SLOTS // 4):
                    ts_v, ts_t = [], []
                    for ci in range(4):
                        c = cq * 4 + ci
                        r = ck * SLOTS + c
                        u = uvpool.tile([128, B], f32, tag="u")
                        nc.scalar.activation(
                            u[:], ga[:, c, :], Ident,
                            scale=wk[3][:, r:r + 1], bias=wk[2][:, r:r + 1],
                        )
                        v = uvpool.tile([128, B], f32, tag="v")
                        nc.vector.tensor_scalar(
                            v[:], ga[:, c, :],
                            wk[1][:, r:r + 1], wk[0][:, r:r + 1],
                            op0=MULT, op1=ADD,
                        )
                        t = uvpool.tile([128, B], f32, tag="t")
                        eng = nc.gpsimd if ci == 3 else nc.vector
                        eng.tensor_tensor(t[:], u[:], gb[:, c, :], op=MULT)
                        ts_v.append(v)
                        ts_t.append(t)
                    for h in range(PB):
                        po = psumO.tile([128, 512], f32, tag="po")
                        for ci in range(4):
                            hs = slice(h * 128, (h + 1) * 128)
                            nc.tensor.matmul(
                                po[:, ci * 128:(ci + 1) * 128],
                                ts_t[ci][:, hs], ident[:],
                                is_transpose=True, start=True, stop=False,
                            )
                            nc.tensor.matmul(
                                po[:, ci * 128:(ci + 1) * 128],
                                ts_v[ci][:, hs], ident[:],
                                is_transpose=True, start=False, stop=True,
                            )
                        og = ostg.tile([128, 512], f32, tag="og")
                        if (h + cq) % 2 == 0:
                            nc.vector.tensor_copy(og[:], po[:])
                        else:
                            nc.scalar.copy(og[:], po[:])
                        j0 = ck * NJ + cq * 512
                        nc.sync.dma_start(
                            out[h * 128:(h + 1) * 128, j0:j0 + 512], og[:]
                        )
    nc.compile()
    return nc


# ---------------------------------------------------------------- host side
def _wrap_idx(idx, OUT, NJ):
    """Pack an index vector into dma_gather's wrapped int16 layout.

    Per chunk ck the NJ indices live in columns [ck*NJ/16, (ck+1)*NJ/16):
    idx16[p, ck*NJ/16 + s] = idx[ck*NJ + s*16 + p%16], replicated over the
    8 groups of 16 partitions.
    """
    nch = OUT // NJ
    a = np.asarray(idx).astype(np.int16).reshape(nch, NJ // 16, 16)  # [ck, s, p]
    a = a.transpose(2, 0, 1).reshape(16, nch * (NJ // 16))           # [p, ck*s]
    return np.ascontiguousarray(np.tile(a, (8, 1)))                  # [128, ...]


def _prep_inputs(x, weights, idx_a, idx_b, NJ=1024):
    B = B_TOT // NCORES
    NCH = OUT_DIM // NJ
    SLOTS = NJ // 128
    x = np.asarray(x, dtype=np.float32)
    weights = np.asarray(weights, dtype=np.float32)
    # wgt_shuf[q, (ck*SLOTS+c)*16+k] = weights[ck*NJ + c*128 + q, k]
    wgt_shuf = np.ascontiguousarray(
        weights.reshape(NCH, SLOTS, 128, 16).transpose(2, 0, 1, 3).reshape(128, -1)
    )
    cgate = np.ascontiguousarray(np.tile(GATE_C.T.reshape(1, 64), (128, 1)))
    ia = _wrap_idx(idx_a, OUT_DIM, NJ)
    ib = _wrap_idx(idx_b, OUT_DIM, NJ)
    in_maps = []
    for c in range(NCORES):
        in_maps.append(
            {
                "x": np.ascontiguousarray(x[c * B:(c + 1) * B]),
                "wgt_shuf": wgt_shuf,
                "cgate": cgate,
                "idxa16": ia,
                "idxb16": ib,
            }
        )
    return in_maps


_NC_CACHE = {}


def _get_nc():
    if "nc" not in _NC_CACHE:
        _NC_CACHE["nc"] = build_nc()
    return _NC_CACHE["nc"]


def kernel(x, weights, idx_a, idx_b):
    import sys

    if "/opt/trn_rl_repo" not in sys.path:
        sys.path.insert(0, "/opt/trn_rl_repo")
    from concourse.bass_utils import run_bass_kernel_spmd

    nc = _get_nc()
    in_maps = _prep_inputs(x, weights, idx_a, idx_b)
    res = run_bass_kernel_spmd(nc, in_maps, list(range(NCORES)))
    return np.concatenate([r["out"] for r in res.results], axis=0)


if __name__ == "__main__":
    nc = build_nc()
    print("built OK")



# revision 2
# speedup vs baseline: 6.8879x; 6.8879x over previous
"""Trainium2 Bass kernel for nn_LogicLayer (differentiable logic-gate layer).

Reference computation:
    a = x[:, idx_a]; b = x[:, idx_b]                  # [B, OUT] gathers
    w = softmax(weights, -1)                          # [OUT, 16]
    out = sum_k w[:, k] * gate_k(a, b)

Every gate value is of the form c0 + c1*a + c2*b + c3*a*b, so
    out[i, j] = W0[j] + W1[j]*a + W2[j]*b + W3[j]*a*b
with W = softmax(weights) @ C, C the [16, 4] gate-coefficient table.

Kernel strategy (out_dim-parallel across 8 cores, 1024 outputs/core,
full 2048-row batch per core):
  - host passes xT = x.T as fp16 [8192, 2048]; each gathered row is then
    4 KiB, so a core needs only 2048 gather descriptors total (SWDGE
    descriptor generation at ~8.5 ns/desc was the old bottleneck)
  - softmax+C projection on device -> W0..W3 [128, 8] tiles in SBUF
  - dma_gather rows of xT for idx_a / idx_b; out_dim lands on partitions
    (j = slot*128 + p), batch on the free axis
  - u = W3*a + W2 (ACT), v = W1*a + W0 (DVE ts), t = u*b (DVE tt),
    o = t + v (DVE tt), everything fp16
  - o stores straight to DRAM as outT [1024, 2048] fp16; host transposes
    back to [2048, OUT] f32 and concatenates the 8 core slices
"""

import numpy as np

# ---------------------------------------------------------------- constants
B_TOT, IN_DIM, OUT_DIM = 2048, 8192, 8192
NCORES = 8
OUT_SH = OUT_DIM // NCORES      # 1024 outputs per core
NSLOT = OUT_SH // 128           # 8 partition-slots per core
NJ = 512                        # indices per dma_gather call
NCH = OUT_SH // NJ              # gather chunks per core
CH = NJ // 128                  # slots per chunk

# value = c0 + c1*a + c2*b + c3*ab  for each of the 16 gates
GATE_C = np.array(
    [
        # c0  c1  c2  c3
        [0, 0, 0, 0],    # 0  False
        [0, 0, 0, 1],    # 1  a AND b
        [0, 1, 0, -1],   # 2  a AND NOT b
        [0, 1, 0, 0],    # 3  a
        [0, 0, 1, -1],   # 4  NOT a AND b
        [0, 0, 1, 0],    # 5  b
        [0, 1, 1, -2],   # 6  a XOR b
        [0, 1, 1, -1],   # 7  a OR b
        [1, -1, -1, 1],  # 8  NOT (a OR b)
        [1, -1, -1, 2],  # 9  NOT (a XOR b)
        [1, 0, -1, 0],   # 10 NOT b
        [1, 0, -1, 1],   # 11 a OR NOT b
        [1, -1, 0, 0],   # 12 NOT a
        [1, -1, 0, 1],   # 13 NOT a OR b
        [1, 0, 0, -1],   # 14 NOT (a AND b)
        [1, 0, 0, 0],    # 15 True
    ],
    dtype=np.float32,
)  # [16, 4]


# ---------------------------------------------------------------- device IR
def build_nc(B=B_TOT, IN=IN_DIM, OSH=OUT_SH):
    """Build the per-core Bass module (SPMD; all cores run the same IR)."""
    import sys

    if "/opt/trn_rl_repo" not in sys.path:
        sys.path.insert(0, "/opt/trn_rl_repo")

    import concourse.tile as tile
    from concourse import bacc, mybir
    from contextlib import ExitStack

    f32 = mybir.dt.float32
    f16 = mybir.dt.float16
    i16 = mybir.dt.int16

    nc = bacc.Bacc("TRN2", target_bir_lowering=False)
    xT = nc.declare_dram_parameter("xT", [IN, B], f16, isOutput=False)
    wgt = nc.declare_dram_parameter("wgt_shuf", [128, NSLOT * 16], f32, isOutput=False)
    cg = nc.declare_dram_parameter("cgate", [128, 64], f32, isOutput=False)
    idxa = nc.declare_dram_parameter("idxa16", [128, OSH // 16], i16, isOutput=False)
    idxb = nc.declare_dram_parameter("idxb16", [128, OSH // 16], i16, isOutput=False)
    outT = nc.declare_dram_parameter("outT", [OSH, B], f16, isOutput=True)

    Ident = mybir.ActivationFunctionType.Identity
    Exp = mybir.ActivationFunctionType.Exp
    MULT = mybir.AluOpType.mult
    ADD = mybir.AluOpType.add

    with tile.TileContext(nc) as tc, ExitStack() as ctx:
        cpool = ctx.enter_context(tc.tile_pool(name="consts", bufs=1))
        wpool = ctx.enter_context(tc.tile_pool(name="wtmp", bufs=2))
        gpool = ctx.enter_context(tc.tile_pool(name="gath", bufs=4, side="right"))
        upool = ctx.enter_context(tc.tile_pool(name="u", bufs=3))
        vpool = ctx.enter_context(tc.tile_pool(name="v", bufs=3))
        tpool = ctx.enter_context(tc.tile_pool(name="t", bufs=3))
        opool = ctx.enter_context(tc.tile_pool(name="o", bufs=3))

        # small input loads first so the W-phase chain starts immediately
        cgt = cpool.tile([128, 64], f32, name="cgt")
        nc.sync.dma_start(cgt[:], cg[:])
        wtile = wpool.tile([128, NSLOT * 16], f32, name="wtile")
        nc.sync.dma_start(wtile[:], wgt[:])
        idxa_sb = cpool.tile([128, OSH // 16], i16, name="idxa_sb")
        nc.sync.dma_start(idxa_sb[:], idxa[:])
        idxb_sb = cpool.tile([128, OSH // 16], i16, name="idxb_sb")
        nc.sync.dma_start(idxb_sb[:], idxb[:])

        # ---- gathers launch first (longest dependency chain) -------------
        # j = (ck*CH + c)*128 + p lands at ga[p, c, :]
        gt = {}
        for ck in range(NCH):
            ga = gpool.tile([128, CH, B], f16, name=f"ga{ck}", tag="ga")
            nc.gpsimd.dma_gather(
                ga[:], xT[:], idxa_sb[:, ck * (NJ // 16):(ck + 1) * (NJ // 16)],
                NJ, NJ, B,
            )
            gb = gpool.tile([128, CH, B], f16, name=f"gb{ck}", tag="gb")
            nc.gpsimd.dma_gather(
                gb[:], xT[:], idxb_sb[:, ck * (NJ // 16):(ck + 1) * (NJ // 16)],
                NJ, NJ, B,
            )
            gt[ck] = (ga, gb)

            # ---- W = softmax(weights) @ C, layout wk[k][q, r], j = r*128+q
            # (issued after the first chunk's gathers so SWDGE gen leads)
            if ck == 0:
                wexp = wpool.tile([128, NSLOT * 16], f32, name="wexp")
                nc.scalar.activation(wexp[:], wtile[:], Exp)
                wsum = wpool.tile([128, NSLOT], f32, name="wsum")
                nc.vector.tensor_reduce(
                    out=wsum[:],
                    in_=wexp[:].rearrange("p (r k) -> p r k", k=16),
                    op=ADD,
                    axis=mybir.AxisListType.X,
                )
                wrcp = wpool.tile([128, NSLOT], f32, name="wrcp")
                nc.vector.reciprocal(wrcp[:], wsum[:])
                wk = [cpool.tile([128, NSLOT], f32, name=f"wk{k}") for k in range(4)]
                for k in range(4):
                    wtmp = wpool.tile([128, NSLOT * 16], f32, name="wtmp", tag="wtmp")
                    ck_bcast = (
                        cgt[:, k * 16:(k + 1) * 16]
                        .rearrange("p (r k) -> p r k", r=1)
                        .to_broadcast([128, NSLOT, 16])
                    )
                    nc.vector.tensor_tensor(
                        out=wtmp[:].rearrange("p (r k) -> p r k", k=16),
                        in0=wexp[:].rearrange("p (r k) -> p r k", k=16),
                        in1=ck_bcast,
                        op=MULT,
                    )
                    wred = wpool.tile([128, NSLOT], f32, name="wred", tag="wred")
                    nc.vector.tensor_reduce(
                        out=wred[:],
                        in_=wtmp[:].rearrange("p (r k) -> p r k", k=16),
                        op=ADD,
                        axis=mybir.AxisListType.X,
                    )
                    nc.vector.tensor_tensor(out=wk[k][:], in0=wred[:], in1=wrcp[:],
                                            op=MULT)

        # ---- gates: out = (W1*a + W0) + (W3*a + W2)*b --------------------
        for ck in range(NCH):
            ga, gb = gt[ck]
            for c in range(CH):
                r = ck * CH + c
                u = upool.tile([128, B], f16, tag="u")
                nc.scalar.activation(
                    u[:], ga[:, c, :], Ident,
                    scale=wk[3][:, r:r + 1], bias=wk[2][:, r:r + 1],
                )
                v = vpool.tile([128, B], f16, tag="v")
                nc.vector.tensor_scalar(
                    v[:], ga[:, c, :],
                    wk[1][:, r:r + 1], wk[0][:, r:r + 1],
                    op0=MULT, op1=ADD,
                )
                t = tpool.tile([128, B], f16, tag="t")
                nc.vector.tensor_tensor(t[:], u[:], gb[:, c, :], op=MULT)
                o = opool.tile([128, B], f16, tag="o")
                nc.vector.tensor_tensor(o[:], t[:], v[:], op=ADD)
                nc.sync.dma_start(outT[r * 128:(r + 1) * 128, :], o[:])
    nc.compile()
    return nc


# ---------------------------------------------------------------- host side
def _wrap_idx(idx, n, NJ=NJ):
    """Pack an index vector into dma_gather's wrapped int16 layout.

    Per chunk ck the NJ indices live in columns [ck*NJ/16, (ck+1)*NJ/16):
    idx16[p, ck*NJ/16 + s] = idx[ck*NJ + s*16 + p%16], replicated over the
    8 groups of 16 partitions.
    """
    nch = n // NJ
    a = np.asarray(idx).astype(np.int16).reshape(nch, NJ // 16, 16)  # [ck, s, p]
    a = a.transpose(2, 0, 1).reshape(16, nch * (NJ // 16))           # [p, ck*s]
    return np.ascontiguousarray(np.tile(a, (8, 1)))                  # [128, ...]


def _prep_inputs(x, weights, idx_a, idx_b):
    x = np.asarray(x, dtype=np.float32)
    weights = np.asarray(weights, dtype=np.float32)
    idx_a = np.asarray(idx_a)
    idx_b = np.asarray(idx_b)
    xT16 = np.ascontiguousarray(x.astype(np.float16).T)  # [IN, B] fp16
    cgate = np.ascontiguousarray(np.tile(GATE_C.T.reshape(1, 64), (128, 1)))
    in_maps = []
    for c in range(NCORES):
        j0 = c * OUT_SH
        wsh = weights[j0:j0 + OUT_SH]  # [1024, 16]
        # wgt_shuf[q, r*16+k] = weights[j0 + r*128 + q, k]
        wgt_shuf = np.ascontiguousarray(
            wsh.reshape(NSLOT, 128, 16).transpose(1, 0, 2).reshape(128, -1)
        )
        in_maps.append(
            {
                "xT": xT16,
                "wgt_shuf": wgt_shuf,
                "cgate": cgate,
                "idxa16": _wrap_idx(idx_a[j0:j0 + OUT_SH], OUT_SH),
                "idxb16": _wrap_idx(idx_b[j0:j0 + OUT_SH], OUT_SH),
            }
        )
    return in_maps


def _assemble(results):
    """[OUT_SH, B] fp16 per core -> full [B, OUT] f32."""
    stacked = np.stack([np.asarray(r["outT"]) for r in results])  # [8, 1024, 2048]
    return np.ascontiguousarray(
        stacked.astype(np.float32).transpose(2, 0, 1).reshape(B_TOT, OUT_DIM)
    )


_NC_CACHE = {}


def _get_nc():
    if "nc" not in _NC_CACHE:
        _NC_CACHE["nc"] = build_nc()
    return _NC_CACHE["nc"]


def kernel(x, weights, idx_a, idx_b):
    import sys

    if "/opt/trn_rl_repo" not in sys.path:
        sys.path.insert(0, "/opt/trn_rl_repo")
    from concourse.bass_utils import run_bass_kernel_spmd

    nc = _get_nc()
    in_maps = _prep_inputs(x, weights, idx_a, idx_b)
    res = run_bass_kernel_spmd(nc, in_maps, list(range(NCORES)))
    return _assemble(res.results)


if __name__ == "__main__":
    nc = build_nc()
    print("built OK")
